# revision 34
# baseline (speedup 1.0000x reference)
"""Trainium2 Bass kernel for nn_Attention_9861244912350.

Fused LayerNorm + QKV projection + block-causal attention + output
projection, sharded over 8 NeuronCores as (batch x head-group):
core c handles batch b = c//2 and heads hg = c%2 (8 of 16 heads).
Each core computes a partial output projection; the host sums the two
half-head partials per batch and adds the output bias.

Key design points (vs a straightforward phase-by-phase version):
- All DRAM inputs (x, weights) are bf16: halves HBM traffic so the LN
  stats / first QKV groups start sooner; every matmul is bf16xbf16
  (same PE stream rate as f32r), accumulation stays fp32 in PSUM.
- LayerNorm application is folded into the QKV projection:
    qkv[j,s] = rstd[s] * ((x @ Wg)[j,s] - mu[s]*c1[j] + std[s]*c2[j])
  with Wg = gamma*W, c1 = sum_d Wg, c2 = beta @ W precomputed on host.
  On device the mu/std correction is one extra rank-2 matmul appended
  to each PSUM accumulation group, and the rstd factor rides the
  PSUM->SBUF evacuation.
- LN statistics matmuls (M=1) are packed with explicit tile_position
  col-tiling (cols 0/32/64/96) so the four sum/sumsq streams of a half
  run concurrently in the PE array on hardware; both halves' matmul
  streams are emitted back-to-back so the halves' row math hides under
  PE work instead of stalling it.
- Attention scores are computed transposed per 128-key tile
  (block-causal => loop bounds, no masking); exp() has no
  max-subtraction; the softmax denominator comes from an augmented
  ones-column in V. The j-loop is software-pipelined (scores j+1
  emitted before PV j). The two per-head-pair score matmuls sit on
  disjoint PE row groups (K=64 at partitions 0/64) so they overlap.
- Softmax normalization is per window pair with NO DRAM bounce: the
  two staged denominator rows are reciprocal'd in SBUF, a rank-2
  matmul (mask lhsT) broadcasts them to 128 partitions in PSUM, and
  the normalize is one DVE multiply off PSUM. Each 512-query slice of
  attnT finalizes as soon as its window pair completes.
- qkT, attnT, V and all probs are bf16; outputs bf16.
- PE never idles across phase seams: QK projections for head-pair
  hp+1 are fed into the PE stream during hp's (Act-bound) attention;
  during hp=3 (no QK work left) output-projection groups are
  interleaved instead, each 512-query slice as soon as its attnT rows
  are normalized; the remainder runs as phase 4.
"""

import numpy as np

B, S, D = 4, 2048, 1024
H, DH, NPATCH = 16, 64, 256
NW = S // NPATCH        # 8 query windows of 256
HL = H // 2             # 8 local heads per core
IL = HL * DH            # 512 local inner dim
NCH = D // 128          # 8 partition chunks of the model dim
KCH = IL // 128         # 4 partition chunks of the local inner dim
NT = S // 128           # 16 key tiles of 128
EPS = 1e-5
SCALE = DH ** -0.5      # 0.125

_STATE = {}


def _build_nc():
    import concourse.bass as bass
    import concourse.mybir as mybir
    import concourse.tile as tile
    from concourse import bacc

    f32 = mybir.dt.float32
    f32r = mybir.dt.float32r
    bf16 = mybir.dt.bfloat16
    AF = mybir.ActivationFunctionType
    OP = mybir.AluOpType

    nc = bacc.Bacc("TRN2", target_bir_lowering=False, debug=False)

    # DRAM I/O (host pre-rearranged so every DMA is contiguous per partition)
    xr = nc.dram_tensor("xr", [128, NCH, S], bf16, kind="ExternalInput")
    wqk = nc.dram_tensor("wqk", [128, 8, NCH, 128], bf16, kind="ExternalInput")
    wv = nc.dram_tensor("wv", [128, NCH, IL], bf16, kind="ExternalInput")
    wo = nc.dram_tensor("wo", [128, KCH, D], bf16, kind="ExternalInput")
    oneD = nc.dram_tensor("oneD", [128, 1], bf16, kind="ExternalInput")
    qkc = nc.dram_tensor("qkc", [2, 8, 128], bf16, kind="ExternalInput")
    vcc = nc.dram_tensor("vcc", [2, IL], bf16, kind="ExternalInput")
    outT = nc.dram_tensor("outT", [128, 8, S], bf16, kind="ExternalOutput")
    # DRAM bounce scratch for the rstd partition broadcasts (internal DRAM
    # pools fail NEFF load under the axon PJRT path, so use an output
    # tensor instead): rows 0-3 hold rstd (2048 vals).
    scr = nc.dram_tensor("scr", [4, 512], f32, kind="ExternalOutput")

    def mm(out, lhsT, rhs, **kw):
        nc.tensor.matmul(out, lhsT, rhs, **kw)

    with tile.TileContext(nc) as tc:
        from contextlib import ExitStack

        with ExitStack() as ctx:
            pconst = ctx.enter_context(tc.tile_pool(name="pconst", bufs=1))
            # One long-lived pool; big buffers share slots via tags:
            #   xlo: x chunks 0-3                   (16 KiB)
            #   xhi: x chunks 4-7  -> wo_sb         (16 KiB)
            #   qkT: Q^T/K^T                        (32 KiB)
            #   vau: V (+ones col)                  (16.5 KiB)
            #   attnT                               (16 KiB)
            pbig = ctx.enter_context(tc.tile_pool(name="pbig", bufs=1))

            oD = pconst.tile([128, 1], bf16)  # 1/D column for stats matmuls
            # correction-row operands replicated at partitions 0/32/64/96 so
            # consecutive K=2 correction matmuls can row-tile and overlap
            qkc_sb = pconst.tile([98, 8, 128], bf16)
            vc_sb = pconst.tile([98, IL], bf16)
            nc.scalar.dma_start(oD, oneD.ap())
            for rp in range(4):
                nc.scalar.dma_start(qkc_sb[32 * rp:32 * rp + 2, :, :],
                                    qkc.ap())
                nc.scalar.dma_start(vc_sb[32 * rp:32 * rp + 2, :], vcc.ap())

            xlo = pbig.tile([128, 4, S], bf16, tag="xlo")
            xhi = pbig.tile([128, 4, S], bf16, tag="xhi")
            for qs in (0, 1):   # quarter pairs: stats half 0 then half 1
                for c in range(NCH):
                    dst = xlo[:, c, :] if c < 4 else xhi[:, c - 4, :]
                    sl = slice(qs * 1024, (qs + 1) * 1024)
                    nc.sync.dma_start(dst[:, sl], xr.ap()[:, c, sl])

            def xc(c):
                return xlo[:, c, :] if c < 4 else xhi[:, c - 4, :]

            # prow spans phases 1+2 only (R/rstd dead once QKV is done).
            # pw/pwv are created BEFORE the phase-1 scratch pools so the
            # weight-load DMAs don't inherit a false dependency on the row
            # tiles' SBUF space being freed.
            pctx = ExitStack()
            prow = pctx.enter_context(tc.tile_pool(name="prow", bufs=1))
            pw = pctx.enter_context(tc.tile_pool(name="pw", bufs=2))
            vctx = ExitStack()
            pwv = vctx.enter_context(tc.tile_pool(name="pwv", bufs=1))
            # first two QK weight tiles, then V weights — all land before
            # the DMA engines get busy with the x loads' tail
            wt_pre = {}
            for tqk in (0, 4):
                wt = pw.tile([128, NCH, 128], bf16, tag="wt")
                nc.sync.dma_start(wt, wqk.ap()[:, tqk, :, :])
                wt_pre[tqk] = wt
            wv_sb = pwv.tile([128, NCH, IL], bf16)
            nc.sync.dma_start(wv_sb, wv.ap())
            vaug = pbig.tile([128, NT, IL], bf16, tag="vau")

            # mu/std rows for rank-2 LN-correction matmuls; rstd broadcasts.
            # One tile per 512-quarter so nothing waits on later quarters.
            Rq = [prow.tile([98, 512], bf16, tag=f"r{n}", name=f"Rq{n}")
                  for n in range(4)]
            rbq = [prow.tile([128, 512], f32, tag=f"bc{n}", name=f"rbq{n}")
                   for n in range(4)]
            rstdcol = prow.tile([128, NT], f32)   # rstd[s] per (s%128, s//128)

            dums = pconst.tile([1, 1], f32)
            nc.gpsimd.memset(dums, 1.0)
            nc.scalar.activation(dums, dums, AF.Sqrt)  # preload Sqrt table
            # ones row for the softmax-normalize broadcast matmuls (K=1,
            # M=64, col-tiled to positions 0/64 so the head-even/odd pair
            # runs concurrently in the PE array)
            E1 = pconst.tile([1, 64], bf16)
            nc.gpsimd.memset(E1, 1.0)
            onesK = pconst.tile([128, 1], bf16)  # denominator matmul lhsT
            nc.gpsimd.memset(onesK, 1.0)

            # ---------------- Phase 1: LN statistics ----------------------
            # The four accumulation streams of a half (sum/sumsq x 2
            # quarters, M=1 each) are col-tiled to PE columns 0/32/64/96 so
            # they run concurrently on HW, and share one 2-bank PSUM tile.
            lctx = ExitStack()
            prows = lctx.enter_context(tc.tile_pool(name="prows", bufs=2))
            psq = lctx.enter_context(tc.tile_pool(name="psq", bufs=4))
            pqctx = ExitStack()
            pqkv = pqctx.enter_context(
                tc.tile_pool(name="pqkv", bufs=4, space="PSUM"))
            scr_h = scr.ap()

            def stats_mms(half, pstat):
                # rows 0/32: s1/s2 of quarter n2=0; rows 64/96: of n2=1
                s12 = pstat.tile([128, 2, 512], f32, tag="s12",
                                 name=f"s12_{half}")
                for c in range(NCH):
                    sqs = []
                    for n2 in range(2):
                        n = 2 * half + n2
                        sl = slice(n * 512, (n + 1) * 512)
                        sq = psq.tile([128, 512], bf16, tag="sq")
                        eng = (nc.vector if (c * 2 + n2) % 2 == 0
                               else nc.gpsimd)
                        eng.tensor_mul(sq, xc(c)[:, sl], xc(c)[:, sl])
                        sqs.append((sq, sl))
                    for n2 in range(2):
                        sq, sl = sqs[n2]
                        p1, p2 = 64 * n2, 64 * n2 + 32
                        mm(s12[p1:p1 + 1, n2, :], oD, xc(c)[:, sl],
                           start=(c == 0), stop=(c == NCH - 1),
                           tile_position=(0, p1))
                        mm(s12[p2:p2 + 1, n2, :], oD, sq,
                           start=(c == 0), stop=(c == NCH - 1),
                           tile_position=(0, p2))
                return s12

            def stats_rowmath(half, s12):
                for n2 in range(2):
                    n = 2 * half + n2
                    s1 = s12[64 * n2:64 * n2 + 1, n2, :]
                    s2 = s12[64 * n2 + 32:64 * n2 + 33, n2, :]
                    tq = prows.tile([1, 512], f32, tag="t")
                    vq = prows.tile([1, 512], f32, tag="t2")
                    # mu -> SBUF first via Act (Pool can't touch PSUM;
                    # TensorTensor may read at most one PSUM input, so
                    # mu^2 squares the SBUF copy)
                    nc.scalar.copy(Rq[n][0:1, :], s1)
                    muSB = Rq[n][0:1, :]
                    nc.vector.tensor_mul(tq, muSB, muSB)
                    nc.scalar.activation(vq, s2, AF.Copy, bias=EPS)
                    nc.vector.tensor_sub(vq, vq, tq)         # ve
                    nc.scalar.activation(vq, vq, AF.Sqrt)    # std
                    # std row -> bf16 at partition 0, then DMA to Rq row 1
                    # (engine writes can't start at partition 1; DMAs can)
                    sb = prows.tile([1, 512], bf16, tag="t3")
                    nc.vector.tensor_copy(sb, vq)
                    nc.scalar.dma_start(Rq[n][1:2, :], sb)
                    for rp in range(1, 4):
                        nc.scalar.dma_start(Rq[n][32 * rp:32 * rp + 2, :],
                                            Rq[n][0:2, :])
                    nc.vector.reciprocal_approx_fast(
                        out=tq, in_=vq)                      # rstd
                    nc.scalar.dma_start(scr_h[n:n + 1, :], tq)
                    nc.scalar.dma_start(
                        rbq[n],
                        bass.AP(tensor=scr_h.tensor,
                                offset=scr_h.offset + n * 512,
                                ap=[[0, 128], [1, 512]]))
                    nc.sync.dma_start(
                        rstdcol[:, 4 * n:4 * n + 4],
                        bass.AP(tensor=scr_h.tensor,
                                offset=scr_h.offset + n * 512,
                                ap=[[1, 128], [128, 4]]))

            # ---------------- Phase 2: QKV projections --------------------
            # V first, then only hp0's Q/K tiles (tqk 0 and 4); the other
            # head-pairs' QK groups are fed into the PE stream DURING the
            # (Act-bound) attention phase of the previous head-pair.
            qkT = pbig.tile([128, 8, S], bf16,
                            tag="qkT")  # t<4: Q^T else K^T

            def qk_mains(tqk, n, wt, pool):
                sl = slice(n * 512, (n + 1) * 512)
                pq = pool.tile([128, 512], f32, tag="pq")
                for c in range(NCH):
                    mm(pq, wt[:, c, :], xc(c)[:, sl],
                       start=(c == 0), stop=False)
                return pq

            def qk_corr(tqk, n, pq, rp):
                # K=2 correction, row-tiled to strip rp so consecutive
                # corrections overlap in the PE array
                pp = 32 * rp
                mm(pq, qkc_sb[pp:pp + 2, tqk, :], Rq[n][pp:pp + 2, :],
                   start=False, stop=True, tile_position=(pp, 0))

            def qk_evac(tqk, n, pq):
                sl = slice(n * 512, (n + 1) * 512)
                nc.vector.tensor_mul(qkT[:, tqk, sl], pq, rbq[n])

            def emit_qk_pack(tqk, ns, wt, pool):
                pqs = [qk_mains(tqk, n, wt, pool) for n in ns]
                for i, n in enumerate(ns):
                    qk_corr(tqk, n, pqs[i], i * (4 // len(ns)))
                for i, n in enumerate(ns):
                    qk_evac(tqk, n, pqs[i])

            def v_mains(st, pool):
                ssl = slice(st * 128, (st + 1) * 128)
                pv = pool.tile([128, 512], f32, tag="pq")
                for c in range(NCH):
                    mm(pv, xc(c)[:, ssl], wv_sb[:, c, :],
                       start=(c == 0), stop=False)
                return pv

            def v_finish(st0, pvs, act_ok=True):
                # 4 corrections row-tiled to strips 0/32/64/96 (concurrent),
                # then the evacuations
                step = 4 // len(pvs)
                for k, pv in enumerate(pvs):
                    st = st0 + k
                    pp = 32 * step * k
                    csl = slice((st % 4) * 128, (st % 4) * 128 + 128)
                    mm(pv, Rq[st // 4][pp:pp + 2, csl],
                       vc_sb[pp:pp + 2, :], start=False, stop=True,
                       tile_position=(pp, 0))
                for k, pv in enumerate(pvs):
                    st = st0 + k
                    dst = vaug[:, st, :]
                    if st % 2 == 0 or not act_ok:
                        nc.vector.tensor_scalar(
                            dst, pv, rstdcol[:, st:st + 1], None, OP.mult)
                    else:
                        nc.scalar.activation(
                            dst, pv, AF.Copy, scale=rstdcol[:, st:st + 1])

            # both halves' stats matmuls stream back-to-back on the PE; the
            # first half's row math (DVE/Act) hides under the second half.
            with tc.tile_pool(name="pstat0", bufs=1, space="PSUM") as ps0, \
                 tc.tile_pool(name="pstat1", bufs=1, space="PSUM") as ps1:
                s12_0 = stats_mms(0, ps0)
                s12_1 = stats_mms(1, ps1)
                stats_rowmath(0, s12_0)
                for st0 in range(0, 8, 2):
                    v_finish(st0, [v_mains(st0, pqkv), v_mains(st0 + 1, pqkv)])
                stats_rowmath(1, s12_1)
            lctx.close()
            nc.scalar.activation(dums, dums, AF.Exp)  # preload Exp table
            v_finish(14, [v_mains(14, pqkv), v_mains(15, pqkv)])

            # only the first half of hp0's Q/K tiles before attention;
            # the n=2,3 packs interleave into hp0's wp0-1 (their queries /
            # keys are first touched at wp2)
            for tqk in (4, 0):
                emit_qk_pack(tqk, (0, 1), wt_pre[tqk], pqkv)
            pqctx.close()  # frees 4 PSUM banks before attention pools open

            # prefetch the output-projection weight now; its slot (xhi)
            # frees once the last interleaved QK group has consumed x
            wo_sb = pbig.tile([128, KCH, D], bf16, tag="xhi")
            nc.scalar.dma_start(wo_sb, wo.ap())

            # ---------------- Phase 3: attention --------------------------
            attnT = pbig.tile([128, KCH, S], bf16, tag="attnT")
            ostate = {"done": 0}
            with ExitStack() as actx:
                pst = actx.enter_context(
                    tc.tile_pool(name="pst", bufs=2, space="PSUM"))
                pos = actx.enter_context(
                    tc.tile_pool(name="pos", bufs=2, space="PSUM"))
                pps = actx.enter_context(
                    tc.tile_pool(name="pps", bufs=2, space="PSUM"))
                ppt = actx.enter_context(tc.tile_pool(name="ppt", bufs=4))
                prr = actx.enter_context(tc.tile_pool(name="prr", bufs=2))
                psm = actx.enter_context(tc.tile_pool(name="psm", bufs=2))
                post = actx.enter_context(tc.tile_pool(name="post", bufs=4))

                def outproj_group(n, tdo, pool, spool, act_ok=False):
                    sl = slice(n * 512, (n + 1) * 512)
                    po = pool.tile([128, 512], f32, tag="pq")
                    for c in range(KCH):
                        mm(po, wo_sb[:, c, tdo * 128:(tdo + 1) * 128],
                           attnT[:, c, sl],
                           start=(c == 0), stop=(c == KCH - 1))
                    out_sb = spool.tile([128, 512], bf16, tag="ost")
                    # during attention the Act engine is saturated by exp,
                    # so interleaved groups evacuate via DVE only; the
                    # post-attention remainder alternates DVE/Act
                    if act_ok and tdo % 2 == 1:
                        nc.scalar.copy(out_sb, po)
                    else:
                        nc.vector.tensor_copy(out_sb, po)
                    deng = (nc.scalar, nc.sync, nc.gpsimd)[tdo % 3]
                    deng.dma_start(outT.ap()[:, tdo, sl], out_sb)

                def build_feeder(hp):
                    # every hp starts with its OWN Q/K n=2,3 packs (first
                    # consumed at its wp2; quota-forced there)
                    items = [
                        lambda tqk=tqk:
                        emit_qk_pack(tqk, (2, 3), wt_pre[tqk], pps)
                        for tqk in (hp + 4, hp)]
                    if hp == 0:
                        for st0 in (8, 10, 12):
                            items.append(
                                lambda st0=st0: v_finish(
                                    st0, [v_mains(st0, pps),
                                          v_mains(st0 + 1, pps)],
                                    act_ok=False))
                    if hp == 3:
                        def mk(i):
                            n, tdo = divmod(i, 8)
                            return lambda: outproj_group(n, tdo, pps, post)
                        return items + [mk(i) for i in range(32)]
                    for tqk in (hp + 1, hp + 5):
                        def mk_load(tqk=tqk):
                            wt = pw.tile([128, NCH, 128], bf16, tag="wt")
                            nc.sync.dma_start(
                                wt, wqk.ap()[:, tqk, :, :])
                            wt_pre[tqk] = wt
                        items.append(mk_load)
                        items.append(
                            lambda tqk=tqk:
                            emit_qk_pack(tqk, (0, 1), wt_pre[tqk], pps))
                    return items

                pre_j0 = [None]   # cross-wp pre-emitted (pt0,) or None
                for hp in range(4):
                    feeder = build_feeder(hp)
                    # hp3's gaps are exp-latency waits; its outproj feeder
                    # items are cheap (4 matmuls), so consume every tick
                    rate = 1 if hp == 3 else 2
                    # next item, tick count, allowed items
                    fstate = [0, 0, 2 if hp == 3 else 99]

                    def tick(force=False):
                        fstate[1] += 1
                        if fstate[0] < min(len(feeder), fstate[2]) and (
                                force or fstate[1] % rate == 0):
                            feeder[fstate[0]]()
                            fstate[0] += 1
                    # per-head-pair: denominator row staging, odd-head O^T
                    # staging, per-wp normalize
                    he, ho = 2 * hp, 2 * hp + 1
                    minq = {2: 4, 3: 5} if hp == 0 else {2: 2}
                    for wp in range(4):
                        while fstate[0] < minq.get(wp, 0):
                            tick(force=True)
                        # window pair (w0, w1): shared key tiles j < 4wp+2
                        # computed once at N=512 for both windows; the two
                        # exclusive tiles (w1 only) at N=256.
                        w0, w1 = 2 * wp, 2 * wp + 1
                        ns = 4 * wp + 2
                        qsl2 = slice(wp * 512, (wp + 1) * 512)
                        # packed PV accumulator: the two heads' PV matmuls
                        # are col-tiled M=64 at array columns 0-63/64-127,
                        # so head-even lands on PSUM partitions 0-63 and
                        # head-odd on 64-127 of ONE bank, and the pair
                        # overlaps in the PE array
                        o_ps = pos.tile([128, 512], f32, tag="ops")
                        # bf16 running sums of the probs tiles (two
                        # independent chains — even j on DVE, odd j on
                        # Pool — so neither serial chain lags the j loop);
                        # the denominator matmuls accumulate over both
                        sm_e = psm.tile([128, 2, 512], bf16, tag="sme")
                        sm_o = psm.tile([128, 2, 512], bf16, tag="smo")
                        sm_init = [False, False]

                        def pv_pair(pj, jprev, stop=False):
                            mm(o_ps[0:64, :],
                               vaug[:, jprev, he * 64:he * 64 + 64],
                               pj[:, 0, :], start=(jprev == 0), stop=stop,
                               tile_position=(0, 0))
                            mm(o_ps[64:128, :],
                               vaug[:, jprev, ho * 64:ho * 64 + 64],
                               pj[:, 1, :], start=(jprev == 0), stop=stop,
                               tile_position=(0, 64))

                        def sm_add(pt, j):
                            eng = nc.vector if j % 2 == 0 else nc.gpsimd
                            sm = sm_e if j % 2 == 0 else sm_o
                            if not sm_init[j % 2]:
                                eng.tensor_copy(sm, pt)
                                sm_init[j % 2] = True
                            else:
                                eng.tensor_add(sm, sm, pt)

                        # software-pipelined: emit scores(j+1) BEFORE PV(j)
                        # so the PE keeps feeding the Act exp stream while
                        # the previous tile's exp is still in flight.
                        prev = None   # (pt, j) awaiting its PV matmuls
                        for j in range(ns):
                            if j == 0 and pre_j0[0] is not None:
                                # scores+exp for this tile were pre-emitted
                                # at the previous wp's seam to keep the Act
                                # exp stream dense
                                prev = (pre_j0[0], 0)
                                sm_add(pre_j0[0], 0)
                                pre_j0[0] = None
                                tick()
                                continue
                            ksl = slice(j * 128, (j + 1) * 128)
                            stp = pst.tile([128, 2, 512], f32, tag="stp")
                            mm(stp[:, 0, :], qkT[0:64, 4 + hp, ksl],
                               qkT[0:64, hp, qsl2], start=True, stop=True)
                            mm(stp[:, 1, :], qkT[64:128, 4 + hp, ksl],
                               qkT[64:128, hp, qsl2], start=True, stop=True)
                            pt = ppt.tile([128, 2, 512], bf16, tag="pt")
                            nc.scalar.activation(pt, stp, AF.Exp, scale=SCALE)
                            sm_add(pt, j)
                            if prev is not None:
                                pv_pair(*prev)
                            prev = (pt, j)
                            tick()
                        # exclusive tiles for w1 (scores first, then the
                        # delayed PV of the last shared tile)
                        stx = pst.tile([128, 2, 512], f32, tag="stp")
                        sxv = stx.rearrange("p a c -> p (a c)").rearrange(
                            "p (a c) -> p a c", c=256)
                        for jj in (0, 1):
                            j = ns + jj
                            ksl = slice(j * 128, (j + 1) * 128)
                            mm(sxv[:, jj, :], qkT[0:64, 4 + hp, ksl],
                               qkT[0:64, hp, w1 * 256:(w1 + 1) * 256],
                               start=True, stop=True)
                            mm(sxv[:, 2 + jj, :], qkT[64:128, 4 + hp, ksl],
                               qkT[64:128, hp, w1 * 256:(w1 + 1) * 256],
                               start=True, stop=True)
                        ptx = ppt.tile([128, 2, 512], bf16, tag="pt")
                        pxv = ptx.rearrange("p a c -> p (a c)").rearrange(
                            "p (a c) -> p a c", c=256)
                        nc.scalar.activation(ptx, stx, AF.Exp, scale=SCALE)
                        # pre-emit the NEXT window pair's first scores tile
                        # and its exp here, so the Act engine rolls straight
                        # from exp(ptx) into the next pair with no bubble
                        # while the PE works through the PV cleanup below.
                        hpn, wpn = (hp, wp + 1) if wp < 3 else (hp + 1, 0)
                        if hpn < 4:
                            if wp == 3:
                                # next head pair: its Q/K n=0 tiles come from
                                # this hp's feeder — emit everything except
                                # the final n=2,3 K pack (not needed by j0)
                                while fstate[0] < len(feeder):
                                    tick(force=True)
                            qsn = slice(wpn * 512, (wpn + 1) * 512)
                            stp0 = pst.tile([128, 2, 512], f32, tag="stp")
                            mm(stp0[:, 0, :], qkT[0:64, 4 + hpn, 0:128],
                               qkT[0:64, hpn, qsn], start=True, stop=True)
                            mm(stp0[:, 1, :], qkT[64:128, 4 + hpn, 0:128],
                               qkT[64:128, hpn, qsn], start=True, stop=True)
                            pt0 = ppt.tile([128, 2, 512], bf16, tag="pt")
                            nc.scalar.activation(pt0, stp0, AF.Exp,
                                                 scale=SCALE)
                            pre_j0[0] = pt0
                        pj, jprev = prev
                        pv_pair(pj, jprev)
                        for jj in (0, 1):
                            j = ns + jj
                            mm(o_ps[0:64, 256:512],
                               vaug[:, j, he * 64:he * 64 + 64],
                               pxv[:, jj, :], start=False, stop=(jj == 1),
                               tile_position=(0, 0))
                            mm(o_ps[64:128, 256:512],
                               vaug[:, j, ho * 64:ho * 64 + 64],
                               pxv[:, 2 + jj, :], start=False, stop=(jj == 1),
                               tile_position=(0, 64))
                        tick()
                        # the denominator matmul below waits for the sum
                        # chains' tail — keep the PE busy on feeder work
                        # (incl. anything the next wp's quota needs)
                        while fstate[0] < minq.get(wp + 1, 0):
                            tick(force=True)
                        tick(force=True)
                        # fold the exclusive probs into the w1 half of the
                        # running sum (both key tiles added per head)
                        tmx = psm.tile([128, 2, 256], bf16, tag="tx")
                        pxh = ptx.rearrange("p a (j c) -> p a j c", c=256)
                        nc.gpsimd.tensor_add(
                            tmx, pxh[:, :, 0, :], pxh[:, :, 1, :])
                        nc.gpsimd.tensor_add(
                            sm_o[:, :, 256:512], sm_o[:, :, 256:512], tmx)
                        # denominators: one col-packed M=1 matmul pair over
                        # the summed probs (head-e -> PSUM partition 0,
                        # head-o -> partition 32)
                        ld = pps.tile([128, 512], f32, tag="pq")
                        mm(ld[0:1, :], onesK, sm_e[:, 0, :], start=True,
                           stop=False, tile_position=(0, 0))
                        mm(ld[32:33, :], onesK, sm_e[:, 1, :], start=True,
                           stop=False, tile_position=(0, 32))
                        mm(ld[0:1, :], onesK, sm_o[:, 0, :], start=False,
                           stop=True, tile_position=(0, 0))
                        mm(ld[32:33, :], onesK, sm_o[:, 1, :], start=False,
                           stop=True, tile_position=(0, 32))
                        L2b = prr.tile([1, 2, 2, 256], f32, tag="lhp")
                        L2v = L2b.rearrange("p i w c -> p i (w c)")
                        nc.vector.tensor_copy(L2v[0:1, 0, :], ld[0:1, :])
                        nc.vector.tensor_copy(L2v[0:1, 1, :], ld[32:33, :])
                        # evacuate the packed accumulator straight into
                        # attnT — partitions are already head-aligned
                        nc.vector.tensor_copy(attnT[:, hp, qsl2], o_ps)
                        # normalize this wp's 512-query slice in place:
                        # tiny reciprocal, rank-2 PE broadcast into PSUM
                        # (rows 0-63 get 1/l_even, 64-127 get 1/l_odd),
                        # one DVE multiply.
                        L2f = L2b.rearrange("p i w c -> p (i w) c")
                        nc.vector.reciprocal_approx_fast(out=L2f, in_=L2f)
                        L2r = prr.tile([1, 2, 512], bf16, tag="lhr")
                        nc.vector.tensor_copy(
                            L2r, L2b.rearrange("p i w c -> p i (w c)"))
                        rbp = pps.tile([128, 512], f32, tag="pq")
                        mm(rbp[0:64, :], E1, L2r[0:1, 0, :],
                           start=True, stop=True, tile_position=(0, 0))
                        mm(rbp[64:128, :], E1, L2r[0:1, 1, :],
                           start=True, stop=True, tile_position=(0, 64))
                        nc.vector.tensor_mul(
                            attnT[:, hp, qsl2], attnT[:, hp, qsl2], rbp)
                        if hp == 3:
                            fstate[2] += 8   # this 512-q outproj slice ok
                    if hp < 3:
                        while fstate[0] < len(feeder):
                            tick(force=True)
                    else:
                        ostate["done"] = max(0, fstate[0] - 2)
            vctx.close()   # wv_sb dead only after hp0's interleaved V pairs

            # ---------------- Phase 4: output projection ------------------
            # (whatever hp=3's interleave didn't get to)
            with ExitStack() as octx:
                post2 = octx.enter_context(tc.tile_pool(name="post2", bufs=4))
                pop = octx.enter_context(
                    tc.tile_pool(name="pop", bufs=4, space="PSUM"))
                for i in range(ostate["done"], 32):
                    n, tdo = divmod(i, 8)
                    outproj_group(n, tdo, pop, post2, act_ok=True)

            pctx.close()

    nc.compile()
    return nc


def _get_nc():
    if "nc" not in _STATE:
        _STATE["nc"] = _build_nc()
    return _STATE["nc"]


def _full_in_maps(x, ln_gamma, ln_beta, Wqkv, Wout):
    import ml_dtypes
    bf16 = ml_dtypes.bfloat16
    x = np.ascontiguousarray(np.asarray(x, np.float32))
    Wq = np.asarray(Wqkv, np.float32)
    Wo = np.asarray(Wout, np.float32)
    g = np.asarray(ln_gamma, np.float32)
    bt = np.asarray(ln_beta, np.float32)
    Wg = Wq * g[:, None]
    c1 = Wg.sum(axis=0)       # [3*D]
    c2 = bt @ Wq              # [3*D]
    in_maps = []
    for c in range(8):
        b, hg = divmod(c, 2)
        xT = x[b].T                                   # [D, S]
        xr = np.ascontiguousarray(
            xT.reshape(NCH, 128, S).transpose(1, 0, 2).astype(bf16))
        qk_idx = np.concatenate(
            [np.arange(hg * IL, (hg + 1) * IL),
             D + np.arange(hg * IL, (hg + 1) * IL)])
        v_idx = 2 * D + np.arange(hg * IL, (hg + 1) * IL)
        wqk_r = np.ascontiguousarray(
            Wg[:, qk_idx].reshape(NCH, 128, 8, 128)
            .transpose(1, 2, 0, 3).astype(bf16))
        wv_r = np.ascontiguousarray(
            Wg[:, v_idx].reshape(NCH, 128, IL)
            .transpose(1, 0, 2).astype(bf16))
        qkc_r = np.ascontiguousarray(
            np.stack([-c1[qk_idx], c2[qk_idx]]).reshape(2, 8, 128)
            .astype(bf16))
        vcc_r = np.ascontiguousarray(
            np.stack([-c1[v_idx], c2[v_idx]]).astype(bf16))
        wo_r = np.ascontiguousarray(
            Wo[hg * IL:(hg + 1) * IL, :]
            .reshape(KCH, 128, D).transpose(1, 0, 2).astype(bf16))
        in_maps.append({
            "xr": xr, "wqk": wqk_r, "wv": wv_r, "wo": wo_r,
            "qkc": qkc_r, "vcc": vcc_r,
            "oneD": np.full((128, 1), 1.0 / D, np.float32).astype(bf16),
        })
    return in_maps


def kernel(x, ln_gamma, ln_beta, Wqkv, Wout, bout):
    from concourse.bass_utils import run_bass_kernel_spmd
    nc = _get_nc()
    bout = np.asarray(bout, np.float32)
    in_maps = _full_in_maps(x, ln_gamma, ln_beta, Wqkv, Wout)
    res = run_bass_kernel_spmd(nc, in_maps, core_ids=list(range(8)))
    _STATE["last_result"] = res
    out = np.empty((B, S, D), np.float32)
    for b in range(B):
        p0 = np.asarray(res.results[2 * b]["outT"], np.float32)
        p1 = np.asarray(res.results[2 * b + 1]["outT"], np.float32)
        partialT = (p0 + p1).transpose(1, 0, 2).reshape(D, S)
        out[b] = partialT.T + bout
    return out


def timed_run(x, ln_gamma, ln_beta, Wqkv, Wout, bout, iters=20):
    """Measure steady-state per-execution time with inputs resident
    on-device (excludes host<->device transfer and compile)."""
    import time
    import jax
    from jax.sharding import Mesh, PartitionSpec
    from jax.experimental.shard_map import shard_map
    from concourse import mybir
    from concourse.bass2jax import (
        _bass_exec_p, install_neuronx_cc_hook, partition_id_tensor)

    install_neuronx_cc_hook()
    nc = _get_nc()
    in_maps = _full_in_maps(x, ln_gamma, ln_beta, Wqkv, Wout)

    pid_name = (nc.partition_id_tensor.name
                if nc.partition_id_tensor is not None else None)
    in_names, out_names, out_avals, zero_outs = [], [], [], []
    for alloc in nc.m.functions[0].allocations:
        if not isinstance(alloc, mybir.MemoryLocationSet):
            continue
        name = alloc.memorylocations[0].name
        if alloc.kind == "ExternalInput":
            if name != pid_name:
                in_names.append(name)
        elif alloc.kind == "ExternalOutput":
            out_names.append(name)
            shape = tuple(alloc.tensor_shape)
            dtype = mybir.dt.np(alloc.dtype)
            out_avals.append(jax.core.ShapedArray(shape, dtype))
            zero_outs.append(np.zeros(shape, dtype))
    n_params = len(in_names)
    all_names = list(in_names) + out_names
    if pid_name is not None:
        all_names.append(pid_name)

    def _body(*args):
        operands = list(args)
        if pid_name is not None:
            operands.append(partition_id_tensor())
        outs = _bass_exec_p.bind(
            *operands,
            out_avals=tuple(out_avals),
            in_names=tuple(all_names),
            out_names=tuple(out_names),
            lowering_input_output_aliases=(),
            sim_require_finite=True,
            sim_require_nnan=True,
            nc=nc,
        )
        return tuple(outs)

    devices = jax.devices()[:8]
    mesh = Mesh(np.asarray(devices), ("core",))
    specs = (PartitionSpec("core"),) * (n_params + len(out_names))
    sharded = jax.jit(
        shard_map(_body, mesh=mesh, in_specs=specs,
                  out_specs=(PartitionSpec("core"),) * len(out_names),
                  check_rep=False),
        keep_unused=True)

    concat_in = [
        np.concatenate([np.asarray(in_maps[c][nm]) for c in range(8)], axis=0)
        for nm in in_names
    ]
    concat_zeros = [
        np.zeros((8 * z.shape[0], *z.shape[1:]), z.dtype) for z in zero_outs
    ]
    sharding = jax.sharding.NamedSharding(mesh, PartitionSpec("core"))
    dev_in = [jax.device_put(a, sharding) for a in concat_in]
    dev_zero = [jax.device_put(a, sharding) for a in concat_zeros]

    out = sharded(*dev_in, *dev_zero)   # warm/compile
    jax.block_until_ready(out)

    def run_n(n):
        t0 = time.monotonic()
        for _ in range(n):
            o = sharded(*dev_in, *dev_zero)
        jax.block_until_ready(o)
        return time.monotonic() - t0

    run_n(2)  # settle
    # the axon tunnel adds large, bursty dispatch noise on top of the real
    # per-iteration execution time; during calm windows the pipelined
    # marginal cost approaches the device execution time. Take the minimum
    # per-trial marginal, but reject heavy-tail flukes (differences far
    # below the median are measurement artifacts, not physics).
    n_lo, n_hi = 6, 6 + iters
    diffs = []
    t_hi_min = None
    for trial in range(24):
        t_lo = run_n(n_lo)
        t_hi = run_n(n_hi)
        t_hi_min = t_hi if t_hi_min is None else min(t_hi, t_hi_min)
        d = (t_hi - t_lo) / (n_hi - n_lo) * 1e9
        if d > 0:
            diffs.append(d)
    if diffs:
        med = sorted(diffs)[len(diffs) // 2]
        valid = [d for d in diffs if d >= 0.35 * med]
        per = min(valid) if valid else med
    else:
        per = t_hi_min / n_hi * 1e9
    return per, {"marginal_ns": per, "avg_ns": t_hi_min / n_hi * 1e9}


def _sim_one_core(core=0):
    """Debug helper: run core `core` through CoreSim against a numpy model."""
    from concourse.bass_interp import CoreSim
    import reference
    inputs = {k: np.asarray(v) for k, v in reference.setup_inputs().items()}
    nc = _get_nc()
    in_maps = _full_in_maps(
        inputs["x"], inputs["ln_gamma"], inputs["ln_beta"],
        inputs["Wqkv"], inputs["Wout"])
    sim = CoreSim(nc, trace=False)
    for k, v in in_maps[core].items():
        sim.tensor(k)[:] = v
    sim.simulate()
    return sim.tensor("outT").copy(), inputs
